# revision 52
# baseline (speedup 1.0000x reference)
"""Trainium2 Bass kernel for nn_CartTensorOut (e3nn-style CartTensorOut layer).

Strategy:
- Data-parallel over nodes: 20000 nodes -> 8 cores x 2500.
- Host folds post_lin vectors P into Wm2 (the [64,9216] MLP weight), which
  collapses the per-node tensor-product + post_lin to an 18-block bilinear
  form; paths 3 and 7 (C111/C221 antisymmetric vs symmetric h x h) vanish,
  and paths 1/5 merge (identical bilinear B).
- Device per chunk of 500 nodes: bf16 matmuls for h/a/wp, DVE pair-products
  of h-planes, PE combine with constant CG matrices, DVE v = wp * B, PE
  reduction to z [6, n]; z DMA'd out.
- Host: sph -> cartesian (Qc) -> segment_sum over sorted batch -> roll.
"""
import numpy as np
import ml_dtypes

BF = ml_dtypes.bfloat16
N_NODES = 20000
N_GRAPH = 256
N_CORES = 8
NC_PER = N_NODES // N_CORES          # 2500
F = 500                               # nodes per chunk
CHUNKS = [500, 500, 500, 500, 500]
NCHUNK = len(CHUNKS)
COFF = [sum(CHUNKS[:i]) for i in range(NCHUNK)]
HC = 32
N_PATH = 9


# ---------------- constant tables (from reference's cartesian tensor algebra) ----
def _tables():
    eps = np.zeros((3, 3, 3))
    eps[0, 1, 2] = eps[1, 2, 0] = eps[2, 0, 1] = 1.0
    eps[0, 2, 1] = eps[2, 1, 0] = eps[1, 0, 2] = -1.0
    s2, s6 = 1 / np.sqrt(2), 1 / np.sqrt(6)
    Q2 = np.zeros((5, 3, 3))
    Q2[0, 0, 1] = Q2[0, 1, 0] = s2
    Q2[1, 1, 2] = Q2[1, 2, 1] = s2
    Q2[2, 0, 0] = Q2[2, 1, 1] = -s6; Q2[2, 2, 2] = 2 * s6
    Q2[3, 0, 2] = Q2[3, 2, 0] = s2
    Q2[4, 0, 0] = s2; Q2[4, 1, 1] = -s2

    def nrm(C, l3):
        return C * np.sqrt((2 * l3 + 1) / (C ** 2).sum())

    M = np.einsum('iab,jdb->ijad', Q2, Q2)
    S = 0.5 * (M + np.transpose(M, (0, 1, 3, 2)))
    S = S - np.trace(S, axis1=2, axis2=3)[..., None, None] * np.eye(3) / 3.0
    C222 = nrm(np.einsum('kad,ijad->ijk', Q2, S), 2)
    Qc = np.zeros((3, 3, 9))
    Qc[:, :, 0] = np.eye(3) / np.sqrt(3)
    Qc[:, :, 1:4] = eps / np.sqrt(2)
    Qc[:, :, 4:9] = np.transpose(Q2, (1, 2, 0))
    return C222, Qc


C222, QC = _tables()


# ---------------- host-side weight folding ----------------
def fold_weights(W0e, W1o, W2e, Wm1, bm1, Wm2, bm2, P0, P1, P2):
    f = {}
    W0 = W0e / np.sqrt(128)
    W1 = W1o / np.sqrt(64)
    W2 = W2e / np.sqrt(32)
    z = np.zeros
    f['Wh0'] = W0.astype(np.float32)
    f['Wh101'] = np.block([[W1, z((64, 32))], [z((64, 32)), W1]]).astype(np.float32)
    # WhZS4: K=96 rows [x1m2(64), x2r4(32)] -> outputs [h1z, s4]
    f['WhZS4'] = np.block([
        [W1, z((64, 32))],
        [z((32, 32)), W2],
    ]).astype(np.float32)
    # WhS: K=128 rows [x2r0..x2r3] -> outputs [s0..s3]
    f['WhS'] = np.block([
        [W2, z((32, 32)), z((32, 32)), z((32, 32))],
        [z((32, 32)), W2, z((32, 32)), z((32, 32))],
        [z((32, 32)), z((32, 32)), W2, z((32, 32))],
        [z((32, 32)), z((32, 32)), z((32, 32)), W2],
    ]).astype(np.float32)
    f['Wm1'] = Wm1.astype(np.float32)
    f['bm1'] = bm1.astype(np.float32).reshape(64, 1)

    W4 = Wm2.reshape(64, N_PATH, HC, HC)
    b4 = bm2.reshape(N_PATH, HC, HC)

    def wpk(k, seg):
        return (W4[:, k] @ seg).astype(np.float32), (b4[k] @ seg).astype(np.float32)

    Wp, Bp = {}, {}
    Wp[0], Bp[0] = wpk(0, P0[0:32])
    Wp[2], Bp[2] = wpk(2, P0[32:64])
    Wp[6], Bp[6] = wpk(6, P0[64:96])
    w1, b1 = wpk(1, P2[0:32])
    w5, b5 = wpk(5, P2[64:96])
    Wp[15], Bp[15] = w1 + w5, b1 + b5
    Wp[4], Bp[4] = wpk(4, P2[32:64])
    Wp[8], Bp[8] = wpk(8, P2[96:128])

    order = [0, 15, 15, 15, 15, 15, 2, 4, 6, 8, 4, 4, 4, 4, 8, 8, 8, 8]
    Wrep = np.concatenate([Wp[k] for k in order], axis=1)          # [64,576]
    brep = np.concatenate([Bp[k] for k in order])[None, :]         # [1,576]
    f['Wm2PrepB'] = np.concatenate([Wrep, brep], axis=0).astype(np.float32)  # [65,576]

    I32 = np.eye(32, dtype=np.float32)

    def mk(K):
        return np.zeros((K, 128), dtype=np.float32)

    s2, s6 = 1 / np.sqrt(2), 1 / np.sqrt(6)
    A2G1 = mk(128)
    for t, c in [(0, 1/np.sqrt(3)), (1, 1/np.sqrt(3)), (2, 1/np.sqrt(3))]:
        A2G1[32*t:32*t+32, 0:32] += c * I32
    for t, c in [(0, -s6), (1, -s6), (2, 2*s6)]:
        A2G1[32*t:32*t+32, 32:64] += c * I32
    A2G1[96:128, 64:96] += (1/np.sqrt(5)) * I32
    A2G1[96:128, 96:128] += C222[4, 4, 2] * I32
    C2G1 = mk(128)
    for i in range(4):
        C2G1[32*i:32*i+32, 64:96] += (1/np.sqrt(5)) * I32
        C2G1[32*i:32*i+32, 96:128] += C222[i, i, 2] * I32
    B2G2 = mk(96)
    B2G2[0:32, 0:32] += 2 * s2 * I32
    B2G2[32:64, 32:64] += 2 * s2 * I32
    B2G2[64:96, 64:96] += 2 * s2 * I32
    SQB2G2 = mk(128)
    SQB2G2[0:32, 96:128] += s2 * I32
    SQB2G2[32:64, 96:128] += -s2 * I32
    D2G3 = mk(128)
    D2G3[32:64, 32:64] += 2 * C222[1, 2, 1] * I32
    D2G3[0:32, 64:96] += 2 * C222[0, 1, 3] * I32
    D2G3[64:96, 64:96] += 2 * C222[2, 3, 3] * I32
    D2G3[96:128, 64:96] += 2 * C222[3, 4, 3] * I32
    E2G3 = mk(64)   # ODE [p02,p13] -> G3 r0
    E2G3[0:32, 0:32] += 2 * C222[0, 2, 0] * I32
    E2G3[32:64, 0:32] += 2 * C222[1, 3, 0] * I32
    F2G3 = mk(96)
    F2G3[0:32, 32:64] += 2 * C222[0, 3, 1] * I32
    F2G3[32:64, 32:64] += 2 * C222[1, 4, 1] * I32
    F2G3[64:96, 96:128] += 2 * C222[2, 4, 4] * I32
    SQ22G3 = mk(128)  # SQ2 diag -> G3 r4
    SQ22G3[32:64, 96:128] += C222[1, 1, 4] * I32
    SQ22G3[96:128, 96:128] += C222[3, 3, 4] * I32
    f.update(A2G1=A2G1, C2G1=C2G1, B2G2=B2G2, SQB2G2=SQB2G2,
             D2G3=D2G3, E2G3=E2G3, F2G3=F2G3, SQ22G3=SQ22G3)

    inv_u = 1.0 / np.sqrt(HC)
    c0 = inv_u / np.sqrt(3 * HC)
    c2 = inv_u / np.sqrt(4 * HC)
    ones = np.ones(32, dtype=np.float32)
    Ra = np.zeros((64, 6), dtype=np.float32)
    Ra[0:32, 0] = c0 * ones
    Ra[32:64, 5] = c2 * ones
    Rb = np.zeros((128, 6), dtype=np.float32)
    for r in range(4):
        Rb[32*r:32*r+32, 1 + r] = c2 * ones
    Rc = np.zeros((128, 6), dtype=np.float32)
    Rc[0:32, 0] = c0 * ones
    Rc[32:64, 3] = c2 * ones
    Rc[64:96, 0] = c0 * ones
    Rc[96:128, 3] = c2 * ones
    Rd = np.zeros((128, 6), dtype=np.float32)
    Rd[0:32, 1] = c2 * ones
    Rd[32:64, 2] = c2 * ones
    Rd[64:96, 4] = c2 * ones
    Rd[96:128, 5] = c2 * ones
    Re = np.zeros((128, 6), dtype=np.float32)
    Re[0:32, 1] = c2 * ones
    Re[32:64, 2] = c2 * ones
    Re[64:96, 4] = c2 * ones
    Re[96:128, 5] = c2 * ones
    f.update(Ra=Ra, Rb=Rb, Rc=Rc, Rd=Rd, Re=Re)
    return f


CONST_NAMES = ['Wh0', 'Wh101', 'WhZS4', 'WhS', 'Wm1', 'Wm2PrepB',
               'A2G1', 'C2G1', 'B2G2', 'SQB2G2', 'D2G3', 'E2G3', 'F2G3',
               'SQ22G3', 'Ra', 'Rb', 'Rc', 'Rd', 'Re']


def pack_consts(f):
    """Pack all bf16 lhsT constants into one [128, W] tensor, zero-padded to
    128 partitions; returns (packed, {name: (k, off, m)})."""
    cols = []
    offs = {}
    w = 0
    for name in CONST_NAMES:
        a = f[name]
        k, m = a.shape
        pad = np.zeros((128, m), dtype=np.float32)
        pad[:k] = a
        cols.append(pad)
        offs[name] = (k, w, m)
        w += m
    return np.concatenate(cols, axis=1).astype(BF), offs


def prep_x(x_scalar, x_spherical):
    """-> xall [128, 5, n] bf16: slot0 = x_scalar.T; slots 1-4 = xsph row-blocks
    [x0(128)] [x1m0,x1m1] [x1m2,x2r0,x2r1] [x2r2,x2r3,x2r4,(pad)]."""
    n = x_scalar.shape[0]
    x1 = x_spherical[:, 128:320].reshape(n, 64, 3)
    x2 = x_spherical[:, 320:480].reshape(n, 32, 5)
    xall = np.zeros((128, 5, n), dtype=np.float32)
    xall[:, 0] = x_scalar.T
    xall[:, 1] = x_spherical[:, 0:128].T
    xall[0:64, 2] = x1[:, :, 0].T
    xall[64:128, 2] = x1[:, :, 1].T
    xall[0:64, 3] = x1[:, :, 2].T
    xall[64:96, 3] = x2[:, :, 4].T
    xall[0:32, 4] = x2[:, :, 0].T
    xall[32:64, 4] = x2[:, :, 1].T
    xall[64:96, 4] = x2[:, :, 2].T
    xall[96:128, 4] = x2[:, :, 3].T
    # chunk-major blocks so each chunk's DMA is one contiguous blob
    blocks = [np.ascontiguousarray(xall[:, :, o:o + f]).reshape(-1)
              for o, f in zip(COFF, CHUNKS)]
    return np.concatenate(blocks).astype(BF)


# ---------------- device program ----------------
_CACHE = {}


def build_program():
    if 'nc' in _CACHE:
        return _CACHE['nc']
    import concourse.bacc as bacc
    from concourse import mybir
    from concourse.tile import TileContext

    bf = mybir.dt.bfloat16
    f32 = mybir.dt.float32
    nc = bacc.Bacc()

    # pack a dummy to get the const layout (offsets depend only on shapes)
    offs = _CACHE['const_offs']
    CW = _CACHE['const_w']

    xall = nc.declare_dram_parameter("xall", [128 * 5 * NC_PER], bf, isOutput=False)
    cpk = nc.declare_dram_parameter("cpk", [128, CW], bf, isOutput=False)
    bm1 = nc.declare_dram_parameter("bm1", [64, 1], f32, isOutput=False)
    zout = nc.declare_dram_parameter("z", [6, NC_PER], f32, isOutput=True)

    with TileContext(nc) as tc:
        with (
            tc.tile_pool(name="consts", bufs=1) as cp,
            tc.tile_pool(name="xin", bufs=2) as xp,
            tc.tile_pool(name="work", bufs=4) as wk,
            tc.tile_pool(name="zpool", bufs=2) as zp,
            tc.tile_pool(name="ps", bufs=8, space="PSUM") as pp,
        ):
            CT = cp.tile([128, CW], bf, name="CT")
            nc.gpsimd.dma_start(out=CT[:], in_=cpk[:])
            bm1_t = cp.tile([64, 1], f32, name="bm1_t")
            nc.gpsimd.dma_start(out=bm1_t[:], in_=bm1[:])

            def W(name):
                k, off, m = offs[name]
                return CT[0:k, off:off + m]

            z_sb = zp.tile([6, NC_PER], f32, name="z_sb")

            for c in range(NCHUNK):
                F = CHUNKS[c]
                sl = slice(COFF[c], COFF[c] + F)
                xoff = 128 * 5 * COFF[c]
                X = xp.tile([128, 5, F], bf, tag="X")
                nc.sync.dma_start(
                    out=X[:],
                    in_=xall[xoff:xoff + 128 * 5 * F].rearrange(
                        "(p s f) -> p s f", p=128, s=5))

                # ---- h / a matmuls (bf16 -> f32 PSUM) ----
                ph0 = pp.tile([32, F], f32, tag="ps")    # [h0]
                phB = pp.tile([128, F], f32, tag="ps")   # [x, y, z, s4]
                phC = pp.tile([128, F], f32, tag="ps")   # [s0..s3]
                pa = pp.tile([64, F], f32, tag="ps")
                nc.tensor.matmul(ph0[:], W('Wh0'), X[:, 1, :], start=True, stop=True)
                nc.tensor.matmul(phB[0:64, :], W('Wh101'), X[:, 2, :], start=True, stop=True,
                                 skip_group_check=True)
                nc.tensor.matmul(phB[64:128, :], W('WhZS4'), X[0:96, 3, :], start=True, stop=True,
                                 skip_group_check=True)
                nc.tensor.matmul(phC[:], W('WhS'), X[:, 4, :], start=True, stop=True)
                nc.tensor.matmul(pa[:], W('Wm1'), X[:, 0, :], start=True, stop=True)

                # ---- evictions (ACT, cast to bf16): one per dst tile ----
                H0R = wk.tile([128, F], bf, tag="H0R")   # [h0 x4]
                H_B = wk.tile([128, F], bf, tag="H_B")   # [x, y, z, s4]
                H2 = wk.tile([128, F], bf, tag="H2")     # [s0, s1, s2, s3]
                nc.scalar.copy(H0R[0:32, :], ph0[:])
                nc.scalar.copy(H_B[:], phB[:])
                nc.scalar.copy(H2[:], phC[:])

                aS = wk.tile([65, F], bf, tag="aS")      # [silu(a); ones]
                nc.scalar.activation(aS[0:64, :], pa[:],
                                     mybir.ActivationFunctionType.Silu,
                                     bias=bm1_t[:], scale=1.0)
                nc.gpsimd.memset(aS[64:65, :], 1.0)

                # ---- shift tiles via engine copies (no DMA issue cost) ----
                nc.gpsimd.tensor_copy(H0R[32:64, :], H0R[0:32, :])
                nc.gpsimd.tensor_copy(H0R[64:128, :], H0R[0:64, :])
                SH_B = wk.tile([96, F], bf, tag="SH_B")    # [y, z, x]
                nc.sync.dma_start(out=SH_B[0:64, :], in_=H_B[32:96, :])
                nc.sync.dma_start(out=SH_B[64:96, :], in_=H_B[0:32, :])
                SH2a = wk.tile([128, F], bf, tag="SH2a")   # [s1, s2, s3, s4]
                nc.gpsimd.tensor_copy(SH2a[0:32, :], H2[32:64, :])
                nc.vector.tensor_copy(SH2a[32:64, :], H2[64:96, :])
                nc.gpsimd.tensor_copy(SH2a[64:96, :], H2[96:128, :])
                nc.vector.tensor_copy(SH2a[96:128, :], H_B[96:128, :])
                SH2b = wk.tile([96, F], bf, tag="SH2b")    # [s3, s4, s4]
                nc.sync.dma_start(out=SH2b[0:32, :], in_=H2[96:128, :])
                nc.sync.dma_start(out=SH2b[32:64, :], in_=H_B[96:128, :])
                nc.sync.dma_start(out=SH2b[64:96, :], in_=H_B[96:128, :])
                SH2c = wk.tile([64, F], bf, tag="SH2c")    # [s2, s3]
                nc.gpsimd.tensor_copy(SH2c[:], H2[64:128, :])

                # ---- pair products (DVE, bf16 2x); v1/v2 inputs first ----
                SM = wk.tile([64, F], bf, tag="SM")        # [h0^2, h0*s4]
                nc.vector.tensor_mul(SM[0:32, :], H0R[0:32, :], H0R[0:32, :])
                nc.vector.tensor_mul(SM[32:64, :], H0R[32:64, :], SH2b[32:64, :])
                HH = wk.tile([128, F], bf, tag="HH")       # [h0*s0..h0*s3]
                nc.vector.tensor_mul(HH[:], H0R[:], H2[:])
                SQB = wk.tile([128, F], bf, tag="SQB")     # [x2,y2,z2,s4^2]
                nc.vector.tensor_mul(SQB[:], H_B[:], H_B[:])
                PRH = wk.tile([96, F], bf, tag="PRH")      # [xy,yz,xz]
                nc.vector.tensor_mul(PRH[:], H_B[0:96, :], SH_B[:])
                SQ2 = wk.tile([128, F], bf, tag="SQ2")     # [s0^2..s3^2]
                nc.vector.tensor_mul(SQ2[:], H2[:], H2[:])
                OD1 = wk.tile([128, F], bf, tag="OD1")     # [p01,p12,p23,p34]
                nc.vector.tensor_mul(OD1[:], H2[:], SH2a[:])
                ODE = wk.tile([64, F], bf, tag="ODE")      # [p02,p13]
                nc.gpsimd.tensor_mul(ODE[:], H2[0:64, :], SH2c[:])
                ODF = wk.tile([96, F], bf, tag="ODF")      # [p03,p14,p24]
                nc.gpsimd.tensor_mul(ODF[:], H2[0:96, :], SH2b[:])

                # ---- wp matmuls (pwa shares a bank with pz) ----
                pwapz = pp.tile([128, F], f32, tag="ps")
                pwa = pwapz[0:64, :]
                pz = pwapz[64:70, :]
                pwb = pp.tile([128, F], f32, tag="ps")
                WP = W('Wm2PrepB')
                nc.tensor.matmul(pwa, WP[:, 0:64], aS[:], start=True, stop=True,
                                 skip_group_check=True)
                nc.tensor.matmul(pwb[:], WP[:, 64:192], aS[:], start=True, stop=True)

                # direct-product v's first: frees pwb early, keeps PSUM peak low
                v1 = wk.tile([64, F], bf, tag="v1")
                v2 = wk.tile([128, F], bf, tag="v2")
                nc.vector.tensor_mul(v1[:], pwa, SM[:])
                nc.vector.tensor_mul(v2[:], pwb[:], HH[:])

                pwc = pp.tile([128, F], f32, tag="ps")
                nc.tensor.matmul(pwc[:], WP[:, 192:320], aS[:], start=True, stop=True)
                pg1 = pp.tile([128, F], f32, tag="ps")
                nc.tensor.matmul(pg1[:], W('A2G1'), SQB[:], start=True, stop=False)
                nc.tensor.matmul(pg1[:], W('C2G1'), SQ2[:], start=False, stop=True)
                G1 = wk.tile([128, F], bf, tag="G1")
                nc.any.tensor_copy(G1[:], pg1[:])
                v3 = wk.tile([128, F], bf, tag="v3")
                nc.vector.tensor_mul(v3[:], pwc[:], G1[:])

                pwd = pp.tile([128, F], f32, tag="ps")
                nc.tensor.matmul(pwd[:], WP[:, 320:448], aS[:], start=True, stop=True)
                pg2 = pp.tile([128, F], f32, tag="ps")
                nc.tensor.matmul(pg2[:], W('B2G2'), PRH[:], start=True, stop=False)
                nc.tensor.matmul(pg2[:], W('SQB2G2'), SQB[:], start=False, stop=True)
                G2 = wk.tile([128, F], bf, tag="G2")
                nc.any.tensor_copy(G2[:], pg2[:])
                v4 = wk.tile([128, F], bf, tag="v4")
                nc.vector.tensor_mul(v4[:], pwd[:], G2[:])

                pwe = pp.tile([128, F], f32, tag="ps")
                nc.tensor.matmul(pwe[:], WP[:, 448:576], aS[:], start=True, stop=True)
                pg3 = pp.tile([128, F], f32, tag="ps")
                nc.tensor.matmul(pg3[:], W('D2G3'), OD1[:], start=True, stop=False)
                nc.tensor.matmul(pg3[:], W('E2G3'), ODE[:], start=False, stop=False)
                nc.tensor.matmul(pg3[:], W('F2G3'), ODF[:], start=False, stop=False)
                nc.tensor.matmul(pg3[:], W('SQ22G3'), SQ2[:], start=False, stop=True)
                G3 = wk.tile([128, F], bf, tag="G3")
                nc.any.tensor_copy(G3[:], pg3[:])
                v5 = wk.tile([128, F], bf, tag="v5")
                nc.vector.tensor_mul(v5[:], pwe[:], G3[:])

                # ---- R reduction ----
                nc.tensor.matmul(pz, W('Ra'), v1[:], start=True, stop=False,
                                 skip_group_check=True)
                nc.tensor.matmul(pz, W('Rb'), v2[:], start=False, stop=False,
                                 skip_group_check=True)
                nc.tensor.matmul(pz, W('Rc'), v3[:], start=False, stop=False,
                                 skip_group_check=True)
                nc.tensor.matmul(pz, W('Rd'), v4[:], start=False, stop=False,
                                 skip_group_check=True)
                nc.tensor.matmul(pz, W('Re'), v5[:], start=False, stop=True,
                                 skip_group_check=True)
                nc.any.tensor_copy(z_sb[:, sl], pz)
                nc.sync.dma_start(out=zout[:, sl], in_=z_sb[:, sl])

    nc.finalize()
    _CACHE['nc'] = nc
    return nc


def kernel(x_scalar, x_spherical, batch, W0e, W1o, W2e, Wm1, bm1, Wm2, bm2,
           P0, P1, P2):
    from concourse.bass_utils import run_bass_kernel_spmd
    import os

    x_scalar = np.asarray(x_scalar, dtype=np.float32)
    x_spherical = np.asarray(x_spherical, dtype=np.float32)
    batch = np.asarray(batch)
    f = fold_weights(np.asarray(W0e, np.float32), np.asarray(W1o, np.float32),
                     np.asarray(W2e, np.float32), np.asarray(Wm1, np.float32),
                     np.asarray(bm1, np.float32), np.asarray(Wm2, np.float32),
                     np.asarray(bm2, np.float32), np.asarray(P0, np.float32),
                     np.asarray(P1, np.float32), np.asarray(P2, np.float32))
    cpk, offs = pack_consts(f)
    _CACHE['const_offs'] = offs
    _CACHE['const_w'] = cpk.shape[1]

    nc = build_program()
    in_maps = []
    for c in range(N_CORES):
        sl = slice(c * NC_PER, (c + 1) * NC_PER)
        xa = prep_x(x_scalar[sl], x_spherical[sl])
        in_maps.append({"xall": xa, "cpk": cpk,
                        "bm1": f['bm1'].astype(np.float32)})

    trace = bool(int(os.environ.get("KERNEL_TRACE", "0")))
    res = run_bass_kernel_spmd(nc, in_maps, core_ids=list(range(N_CORES)),
                               trace=trace)
    _CACHE['last_results'] = res

    # host post-processing: sph -> cart -> segment_sum -> roll
    z = np.concatenate([np.asarray(r["z"], np.float64) for r in res.results],
                       axis=1)                       # [6, 20000]
    sph = np.zeros((N_NODES, 9), dtype=np.float64)
    sph[:, 0] = z[0]
    sph[:, 4:9] = z[1:6].T
    cart = np.einsum('abi,ni->nab', QC, sph)
    red = np.zeros((N_GRAPH, 3, 3), dtype=np.float64)
    np.add.at(red, batch.astype(np.int64), cart)
    out = np.roll(np.roll(red, 1, axis=1), 1, axis=2)
    return out.astype(np.float32)
